# revision 48
# baseline (speedup 1.0000x reference)
"""Trainium2 Bass kernel for grouped (neighborhood) multi-head attention.

Problem: B=2, N=8192, D=512, H=8 heads (d_k=64), K=32 neighbors/node.
  Q/K/V = x @ W{q,k,v}.T ; per-head LayerNorm on Q,K ; gather K,V rows at
  idx[n,k]; softmax(QK/sqrt(dk)) ; out = attn@Vg ; out @ Wout.T + bout.

Sharding (8 cores): core c owns batch b=c//4, node quarter q=c%4 (2048
nodes). The dominant cost in this axon-tunneled setup is host<->device
wire time (~40 MB/s), so the wire format is fp16 and minimal:
  - x uploaded fp16 in natural [n, d] layout (device DMA-transposes it),
  - weights uploaded once-sharded (256 rows/core of the stacked
    [WqT;WkT;WvT;WoT]) and AllGathered on device,
  - no zero-filled output upload (the kernel fully writes its output),
  - output downloaded fp16.
Device: fp32 matmul accumulation, per-head LN, fp16 K|V AllGathered
within each 4-core batch group, indirect-DMA gather of 2KB K|V rows,
scores/softmax/AV on the Vector engine, PE-transpose + fp16 matmul
out-projection.
"""

import sys

sys.path.insert(0, "/opt/trn_rl_repo")

import numpy as np
from contextlib import ExitStack

import concourse.bass as bass
import concourse.mybir as mybir
import concourse.tile as tile
from concourse import bacc
from concourse.bass import ts
from concourse.masks import make_identity

F32 = mybir.dt.float32
F16 = mybir.dt.float16
I32 = mybir.dt.int32
I16 = mybir.dt.int16
I8 = mybir.dt.int8

H = 8
DK = 64
D = 512
KN = 32
B = 2
NCORES = 8
LN_EPS = 1e-5
DCH = D // 128          # contraction chunks (4)
NB = 8192               # nodes per batch
NSH = NB // (NCORES // B)  # nodes per core (2048)


def build_nc(KG=16):
    """Build the SPMD Bass program. KG = neighbor group size.

    Sharding: core c owns node range [1024c, 1024c+1024) of BOTH batches
    (x_sh rows 0:1024 = batch 0, rows 1024:2048 = batch 1). Both batches
    share the neighbor lists, so each core's idx shard is exactly its own
    node range — uploaded once, no cross-core idx exchange needed.
    """
    T = NSH // 128          # node tiles per core (16; 8 per batch)
    TB = T // B             # node tiles per batch (8)
    G = KN // KG            # neighbor groups
    NBC = NSH // B          # nodes per batch per core (1024)
    WR = 4 * D // NCORES    # weight-shard rows (256)
    IR = NBC * KN * 2 // (2 * D)   # idx rows in w_sh (64)
    w_groups = [list(range(NCORES))]

    nc = bacc.Bacc(
        "TRN2", target_bir_lowering=False, debug=False, num_devices=NCORES
    )

    # Packed wire format (minimizes per-transfer round trips):
    #   x_sh cols: [0:512) int8 quantized x | [512:514) f16 row scale
    #   w_sh rows: [0:256) f16 [WqT;WkT;WvT;WoT] shard | row 256 f16 bout
    #              | [257:321) int16 idx for this core's 1024 nodes,
    #              host-pretransformed to kv_full row ids (batch 0)
    #   out_sh cols: [0:512) int8 quantized out | [512:514) f16 row scale
    XC = D + 2
    x_sh = nc.dram_tensor("x_sh", [NSH, XC], I8, kind="ExternalInput")
    w_sh = nc.dram_tensor(
        "w_sh", [WR + 1 + IR, 2 * D], I8, kind="ExternalInput"
    )
    out = nc.dram_tensor("out_sh", [NSH, D + 2], I8, kind="ExternalOutput")

    w_loc = nc.dram_tensor("w_loc", [WR, D], F16)
    wall = nc.dram_tensor("wall", [4 * D, D], F16, addr_space="Shared")
    kv_shard = nc.dram_tensor("kv_shard", [NSH, 2 * D], F16)
    kv_full = nc.dram_tensor("kv_full", [B * NB, 2 * D], F16)

    with ExitStack() as ctx:
        tc = ctx.enter_context(tile.TileContext(nc))
        pconst = ctx.enter_context(tc.tile_pool(name="const", bufs=1))
        poffs = ctx.enter_context(tc.tile_pool(name="offs", bufs=T))
        pq = ctx.enter_context(tc.tile_pool(name="q", bufs=T))
        pao = ctx.enter_context(tc.tile_pool(name="ao", bufs=T))
        pw = ctx.enter_context(tc.tile_pool(name="w", bufs=1))

        ident = pconst.tile([128, 128], F32)
        make_identity(nc, ident[:])
        ident16 = pconst.tile([128, 128], F16)
        make_identity(nc, ident16[:])
        ones_row = pconst.tile([1, 128], F16)
        nc.vector.memset(ones_row[:], 1.0)
        bout_sb = pconst.tile([1, D], F16)
        nc.sync.dma_start(
            out=bout_sb[:],
            in_=w_sh[4 * D // NCORES:4 * D // NCORES + 1, :].bitcast(F16),
        )
        eps_sb = pconst.tile([128, 1], F32)
        nc.vector.memset(eps_sb[:], LN_EPS)

        # idx tiles: 8 per-batch tiles shared by both batches; batch-1
        # offsets are batch-0 offsets + 1024 (kv_full row layout).
        offs_tiles = []
        b1_tiles = []
        for tp in range(TB):
            offs16_t = poffs.tile([128, KN], I16, tag="offs16")
            nc.sync.dma_start(
                out=offs16_t[:],
                in_=w_sh[WR + 1 + 8 * tp: WR + 1 + 8 * (tp + 1), :]
                .bitcast(I16)
                .rearrange("r (s k) -> (r s) k", k=KN),
            )
            offs_t = poffs.tile([128, KN], I32, tag="offs32")
            nc.vector.tensor_copy(out=offs_t[:], in_=offs16_t[:])
            offs_tiles.append(offs_t)
        for tp in range(TB):
            offs_b1 = poffs.tile([128, KN], I32, tag="offs32b1")
            nc.vector.tensor_scalar_add(offs_b1[:], offs_tiles[tp][:], NBC)
            b1_tiles.append(offs_b1)
        offs_tiles.extend(b1_tiles)

        # ------------- Weights: shard -> AllGather -> SBUF --------------
        nc.sync.dma_start(
            out=w_loc[:], in_=w_sh[0:4 * D // NCORES, :].bitcast(F16)
        )
        nc.gpsimd.collective_compute(
            "AllGather",
            mybir.AluOpType.bypass,
            replica_groups=w_groups,
            ins=[w_loc[:]],
            outs=[wall[:]],
        )
        # wall rows: [WqT(512); WkT(512); WvT(512); WoT(512)]
        w_sb = {}
        for mi, wname in enumerate(("q", "k", "v", "o")):
            w_sb[wname] = []
            for dc in range(DCH):
                w_c = pw.tile([128, D], F16, tag=f"w{wname}{dc}")
                nc.sync.dma_start(
                    out=w_c[:], in_=wall[mi * D + dc * 128: mi * D + (dc + 1) * 128, :]
                )
                w_sb[wname].append(w_c)

        q_tiles = []
        ao_tiles = []

        # ---------------- Phase 1: projections + LN + KV shard ----------
        with (
            tc.tile_pool(name="xw", bufs=1) as pxw,
            tc.tile_pool(name="xq", bufs=3) as pxq,
            tc.tile_pool(name="pstx", bufs=4, space="PSUM") as pptx,
            tc.tile_pool(name="ps1", bufs=4, space="PSUM") as pps,
            tc.tile_pool(name="ln", bufs=4) as pln,
        ):
            # int8 x -> f16 (unscaled; LN cancels the per-row scale for Q,K;
            # V gets the scale re-applied below) -> PE-transpose into xT.
            xt_sb = [
                pxw.tile([128, NSH], F16, tag=f"xt{dc}", name=f"xt{dc}")
                for dc in range(DCH)
            ]
            xs_tiles = []
            for t in range(T):
                xq_t = pxq.tile([128, D], I8, tag="xq8")
                nc.sync.dma_start(out=xq_t[:], in_=x_sh[ts(t, 128), 0:D])
                xf_t = pxq.tile([128, D], F16, tag="xf16")
                nc.vector.tensor_copy(out=xf_t[:], in_=xq_t[:])
                for dc in range(DCH):
                    tr = pptx.tile([128, 128], F16, tag="trx")
                    nc.tensor.transpose(
                        out=tr[:], in_=xf_t[:, ts(dc, 128)],
                        identity=ident16[:],
                    )
                    nc.vector.tensor_copy(
                        out=xt_sb[dc][:, ts(t, 128)], in_=tr[:]
                    )
                xs_t = pxw.tile([128, 1], F16, tag=f"xs{t}")
                nc.sync.dma_start(
                    out=xs_t[:],
                    in_=x_sh[ts(t, 128), D:D + 2].bitcast(F16),
                )
                xs_tiles.append(xs_t)

            def layer_norm_from_psum(ps, out_16):
                """Per-head LN of psum tile (128, D) -> fp16 SBUF tile."""
                ps_h = ps[:].rearrange("p (h d) -> p h d", h=H)
                sums = pln.tile([128, H], F32, tag="lnsum")
                nc.vector.tensor_reduce(
                    out=sums[:], in_=ps_h, axis=mybir.AxisListType.X,
                    op=mybir.AluOpType.add,
                )
                sq = pln.tile([128, D], F32, tag="lnsq")
                nc.scalar.square(out=sq[:], in_=ps[:])
                sqs = pln.tile([128, H], F32, tag="lnsqs")
                nc.vector.tensor_reduce(
                    out=sqs[:], in_=sq[:].rearrange("p (h d) -> p h d", h=H),
                    axis=mybir.AxisListType.X, op=mybir.AluOpType.add,
                )
                mu = pln.tile([128, H], F32, tag="lnmu")
                nc.vector.tensor_scalar_mul(mu[:], sums[:], 1.0 / DK)
                var = pln.tile([128, H], F32, tag="lnvar")
                # var = E[x^2] - mu^2   (E[x^2] = sqs/DK)
                nc.vector.tensor_scalar_mul(var[:], sqs[:], 1.0 / DK)
                musq = pln.tile([128, H], F32, tag="lnmusq")
                nc.vector.tensor_tensor(
                    out=musq[:], in0=mu[:], in1=mu[:], op=mybir.AluOpType.mult
                )
                nc.vector.tensor_tensor(
                    out=var[:], in0=var[:], in1=musq[:],
                    op=mybir.AluOpType.subtract,
                )
                std = pln.tile([128, H], F32, tag="lnstd")
                nc.scalar.activation(
                    out=std[:], in_=var[:],
                    func=mybir.ActivationFunctionType.Sqrt, bias=eps_sb[:],
                )
                rstd = pln.tile([128, H], F32, tag="lnrstd")
                nc.vector.reciprocal(rstd[:], std[:])
                cen = pln.tile([128, D], F32, tag="lncen")
                nc.vector.tensor_tensor(
                    out=cen[:].rearrange("p (h d) -> p h d", h=H),
                    in0=ps_h,
                    in1=mu[:].rearrange("p (h o) -> p h o", o=1)
                        .to_broadcast([128, H, DK]),
                    op=mybir.AluOpType.subtract,
                )
                nc.vector.tensor_tensor(
                    out=out_16[:].rearrange("p (h d) -> p h d", h=H),
                    in0=cen[:].rearrange("p (h d) -> p h d", h=H),
                    in1=rstd[:].rearrange("p (h o) -> p h o", o=1)
                        .to_broadcast([128, H, DK]),
                    op=mybir.AluOpType.mult,
                )

            for t in range(T):
                for proj in ("q", "k", "v"):
                    ps = pps.tile([128, D], F32, tag="ps")
                    for dc in range(DCH):
                        nc.tensor.matmul(
                            out=ps[:],
                            lhsT=xt_sb[dc][:, ts(t, 128)],
                            rhs=w_sb[proj][dc][:],
                            start=(dc == 0),
                            stop=(dc == DCH - 1),
                        )
                    if proj == "q":
                        q_t = pq.tile([128, D], F16)
                        layer_norm_from_psum(ps, q_t)
                        q_tiles.append(q_t)
                    elif proj == "k":
                        k_16 = pln.tile([128, D], F16, tag="k16")
                        layer_norm_from_psum(ps, k_16)
                        nc.sync.dma_start(
                            out=kv_shard[ts(t, 128), 0:D], in_=k_16[:]
                        )
                    else:
                        v_16 = pln.tile([128, D], F16, tag="v16")
                        # re-apply the per-row int8 scale (V is linear in x)
                        nc.vector.tensor_tensor(
                            out=v_16[:],
                            in0=ps[:],
                            in1=xs_tiles[t][:].to_broadcast([128, D]),
                            op=mybir.AluOpType.mult,
                        )
                        nc.sync.dma_start(
                            out=kv_shard[ts(t, 128), D:2 * D], in_=v_16[:]
                        )

        # ------------- AllGather K|V across all 8 cores (both batches) --
        nc.gpsimd.collective_compute(
            "AllGather",
            mybir.AluOpType.bypass,
            replica_groups=w_groups,
            ins=[kv_shard[:]],
            outs=[kv_full[:]],
        )

        # ---------------- Phase 2: gather + scores + softmax + AV -------
        with (
            tc.tile_pool(name="kvg", bufs=2) as pkvg,
            tc.tile_pool(name="pbuf", bufs=3) as ppb,
            tc.tile_pool(name="sm", bufs=3) as psm,
        ):
            for t in range(T):
                offs_t = offs_tiles[t]
                kvg_g = []
                for g in range(G):
                    kvg = pkvg.tile([128, KG, 2 * D], F16, tag="kvg")
                    for kk in range(KG):
                        nc.gpsimd.indirect_dma_start(
                            out=kvg[:, kk, :],
                            out_offset=None,
                            in_=kv_full[:],
                            in_offset=bass.IndirectOffsetOnAxis(
                                ap=offs_t[:, g * KG + kk: g * KG + kk + 1],
                                axis=0,
                            ),
                        )
                    kvg_g.append(kvg)

                sc = psm.tile([128, KN, H], F32, tag="sc")
                q_bc = (
                    q_tiles[t][:]
                    .rearrange("p (o h d) -> p o h d", o=1, h=H)
                    .to_broadcast([128, KG, H, DK])
                )
                for g in range(G):
                    pt = ppb.tile([128, KG, H, DK], F16, tag="pbuf")
                    nc.vector.tensor_tensor(
                        out=pt[:],
                        in0=kvg_g[g][:, :, 0:D].rearrange(
                            "p k (h d) -> p k h d", h=H
                        ),
                        in1=q_bc,
                        op=mybir.AluOpType.mult,
                    )
                    # Tree-reduce over d (fp16 to 8 partials, then f32):
                    # cheaper than the 1x TensorReduce on the Vector engine.
                    m = DK // 2
                    while m > 4:
                        nc.vector.tensor_tensor(
                            out=pt[:, :, :, 0:m],
                            in0=pt[:, :, :, 0:m],
                            in1=pt[:, :, :, m:2 * m],
                            op=mybir.AluOpType.add,
                        )
                        m //= 2
                    t8 = psm.tile([128, KG, H, 4], F32, tag="t8", name="t8")
                    nc.vector.tensor_tensor(
                        out=t8[:], in0=pt[:, :, :, 0:4], in1=pt[:, :, :, 4:8],
                        op=mybir.AluOpType.add,
                    )
                    nc.vector.tensor_tensor(
                        out=t8[:, :, :, 0:2], in0=t8[:, :, :, 0:2],
                        in1=t8[:, :, :, 2:4], op=mybir.AluOpType.add,
                    )
                    nc.vector.tensor_tensor(
                        out=sc[:, g * KG:(g + 1) * KG, :]
                            .rearrange("p k (h o) -> p k h o", o=1),
                        in0=t8[:, :, :, 0:1], in1=t8[:, :, :, 1:2],
                        op=mybir.AluOpType.add,
                    )

                # softmax over k (scores bounded by ~8 after LN: skip max)
                es = psm.tile([128, KN, H], F32, tag="es")
                nc.scalar.activation(
                    out=es[:], in_=sc[:],
                    func=mybir.ActivationFunctionType.Exp,
                    scale=1.0 / float(np.sqrt(DK)),
                )
                ssum = psm.tile([128, H], F32, tag="ssum")
                nc.vector.tensor_reduce(
                    out=ssum[:], in_=es[:].rearrange("p k h -> p h k"),
                    axis=mybir.AxisListType.X, op=mybir.AluOpType.add,
                )
                rs = psm.tile([128, H], F32, tag="rs")
                nc.vector.reciprocal(rs[:], ssum[:])
                attn = psm.tile([128, KN, H], F16, tag="attn")
                nc.vector.tensor_tensor(
                    out=attn[:],
                    in0=es[:],
                    in1=rs[:].rearrange("p (o h) -> p o h", o=1)
                        .to_broadcast([128, KN, H]),
                    op=mybir.AluOpType.mult,
                )

                ao_t = pao.tile([128, D], F32)
                ao_tiles.append(ao_t)
                for g in range(G):
                    p2 = ppb.tile([128, KG, H, DK], F16, tag="pbuf")
                    nc.vector.tensor_tensor(
                        out=p2[:],
                        in0=kvg_g[g][:, :, D:2 * D].rearrange(
                            "p k (h d) -> p k h d", h=H
                        ),
                        in1=attn[:, g * KG:(g + 1) * KG, :]
                            .rearrange("p k (h o) -> p k h o", o=1)
                            .to_broadcast([128, KG, H, DK]),
                        op=mybir.AluOpType.mult,
                    )
                    m = KG // 2
                    while m > 1:
                        nc.vector.tensor_tensor(
                            out=p2[:, 0:m],
                            in0=p2[:, 0:m],
                            in1=p2[:, m:2 * m],
                            op=mybir.AluOpType.add,
                        )
                        m //= 2
                    av = psm.tile([128, H, DK], F32, tag="av")
                    nc.vector.tensor_tensor(
                        out=av[:].rearrange("p h d -> p (h d)")
                            .rearrange("p (o h d) -> p o h d", o=1, h=H),
                        in0=p2[:, 0:1],
                        in1=p2[:, 1:2],
                        op=mybir.AluOpType.add,
                    )
                    if g == 0:
                        nc.vector.tensor_copy(
                            out=ao_t[:], in_=av[:].rearrange("p h d -> p (h d)")
                        )
                    else:
                        nc.vector.tensor_tensor(
                            out=ao_t[:],
                            in0=ao_t[:],
                            in1=av[:].rearrange("p h d -> p (h d)"),
                            op=mybir.AluOpType.add,
                        )

        # ---------------- Phase 3: transpose + out-projection -----------
        with (
            tc.tile_pool(name="p3", bufs=1) as p3,
            tc.tile_pool(name="ps3", bufs=4, space="PSUM") as pps3,
            tc.tile_pool(name="pstr", bufs=4, space="PSUM") as pptr,
            tc.tile_pool(name="o3", bufs=3) as po3,
        ):
            aot_sb = [
                p3.tile([128, NSH], F16, tag=f"aot{dc}", name=f"aot{dc}")
                for dc in range(DCH)
            ]
            for t in range(T):
                for dc in range(DCH):
                    tr_ps = pptr.tile([128, 128], F32, tag="tr")
                    nc.tensor.transpose(
                        out=tr_ps[:],
                        in_=ao_tiles[t][:, ts(dc, 128)],
                        identity=ident[:],
                    )
                    nc.vector.tensor_copy(
                        out=aot_sb[dc][:, ts(t, 128)], in_=tr_ps[:]
                    )
            for t in range(T):
                ps = pps3.tile([128, D], F32, tag="ps3")
                for dc in range(DCH):
                    nc.tensor.matmul(
                        out=ps[:],
                        lhsT=aot_sb[dc][:, ts(t, 128)],
                        rhs=w_sb["o"][dc][:],
                        start=(dc == 0),
                        stop=False,
                    )
                nc.tensor.matmul(
                    out=ps[:],
                    lhsT=ones_row[:],
                    rhs=bout_sb[:],
                    start=False,
                    stop=True,
                )
                # int8 per-row quantization: scale = rowmax/127 (f16,
                # downloaded), q = rint(v / scale) via DVE cast-on-write.
                ab = po3.tile([128, D], F32, tag="oab")
                nc.scalar.activation(
                    out=ab[:], in_=ps[:],
                    func=mybir.ActivationFunctionType.Abs,
                )
                amax = po3.tile([128, 1], F32, tag="oamax")
                nc.vector.tensor_reduce(
                    out=amax[:], in_=ab[:], axis=mybir.AxisListType.X,
                    op=mybir.AluOpType.max,
                )
                nc.vector.tensor_scalar_max(amax[:], amax[:], 1e-6)
                s16 = po3.tile([128, 1], F16, tag="os16")
                nc.vector.tensor_scalar_mul(s16[:], amax[:], 1.0 / 127.0)
                r32 = po3.tile([128, 1], F32, tag="or32")
                nc.vector.reciprocal(r32[:], s16[:])
                o_sb = po3.tile([128, D], I8, tag="osb")
                nc.vector.tensor_tensor(
                    out=o_sb[:],
                    in0=ps[:],
                    in1=r32[:].to_broadcast([128, D]),
                    op=mybir.AluOpType.mult,
                )
                nc.sync.dma_start(out=out[ts(t, 128), 0:D], in_=o_sb[:])
                nc.sync.dma_start(
                    out=out[ts(t, 128), D:D + 2].bitcast(F16), in_=s16[:]
                )

    nc.finalize()
    return nc


_RUNNER_CACHE = {}


def _get_runner():
    """Build the Bass program and a cached sharded jit callable that runs
    it on 8 cores via PJRT without uploading zero output buffers."""
    if "runner" in _RUNNER_CACHE:
        return _RUNNER_CACHE["runner"]

    import jax
    from jax.sharding import Mesh, PartitionSpec
    from jax.experimental.shard_map import shard_map
    from concourse.bass2jax import (
        _bass_exec_p,
        install_neuronx_cc_hook,
        partition_id_tensor,
    )

    install_neuronx_cc_hook()
    nc = build_nc()

    partition_name = (
        nc.partition_id_tensor.name if nc.partition_id_tensor else None
    )
    in_names: list[str] = []
    out_names: list[str] = []
    out_avals: list = []
    for alloc in nc.m.functions[0].allocations:
        if not isinstance(alloc, mybir.MemoryLocationSet):
            continue
        name = alloc.memorylocations[0].name
        if alloc.kind == "ExternalInput":
            if name != partition_name:
                in_names.append(name)
        elif alloc.kind == "ExternalOutput":
            assert alloc.tensor_shape is not None and alloc.dtype is not None
            out_names.append(name)
            out_avals.append(
                jax.core.ShapedArray(
                    tuple(alloc.tensor_shape), mybir.dt.np(alloc.dtype)
                )
            )
    n_params = len(in_names)
    bind_names = list(in_names)
    if partition_name is not None:
        bind_names.append(partition_name)

    def _body(*args):
        operands = list(args)
        if partition_name is not None:
            operands.append(partition_id_tensor())
        outs = _bass_exec_p.bind(
            *operands,
            out_avals=tuple(out_avals),
            in_names=tuple(bind_names),
            out_names=tuple(out_names),
            lowering_input_output_aliases=(),
            sim_require_finite=True,
            sim_require_nnan=True,
            nc=nc,
        )
        return tuple(outs)

    devices = jax.devices()[:NCORES]
    mesh = Mesh(np.asarray(devices), ("core",))
    sharding = jax.sharding.NamedSharding(mesh, PartitionSpec("core"))
    _RUNNER_CACHE["devices"] = devices

    in_shapes = {
        "x_sh": ((NCORES * NSH, D + 2), np.int8),
        "w_sh": ((NCORES * (4 * D // NCORES + 1 + 64), 2 * D), np.int8),
    }

    def _make_jit():
        return jax.jit(
            shard_map(
                _body,
                mesh=mesh,
                in_specs=(PartitionSpec("core"),) * len(in_names),
                out_specs=(PartitionSpec("core"),) * len(out_names),
                check_rep=False,
            ),
            keep_unused=True,
        )

    try:
        from concourse.bass2jax import fast_dispatch_compile

        sample = [
            jax.ShapeDtypeStruct(*in_shapes[n], sharding=sharding)
            for n in in_names
        ]
        sharded = fast_dispatch_compile(
            lambda: _make_jit().lower(*sample).compile()
        )
    except Exception:
        sharded = _make_jit()
    _RUNNER_CACHE["runner"] = (sharded, in_names, out_names, sharding)
    return _RUNNER_CACHE["runner"]


def make_global_inputs(x, idx, Wq, Wk, Wv, Wout, bout):
    """Host-side wire format: minimal bytes (see kernel() for layout)."""
    NBC = NSH // B
    xf = np.asarray(x, dtype=np.float32)
    xcomb = np.empty((NCORES * NSH, D + 2), dtype=np.int8)
    for c in range(NCORES):
        rows = slice(c * NBC, (c + 1) * NBC)
        xfc = np.concatenate([xf[0, rows], xf[1, rows]], axis=0)
        rmax = np.abs(xfc).max(axis=1, keepdims=True)
        rmax[rmax == 0] = 1.0
        q = np.multiply(xfc, 127.0 / rmax, dtype=np.float32)
        np.rint(q, out=q)
        xcomb[c * NSH:(c + 1) * NSH, 0:D] = q
        xcomb[c * NSH:(c + 1) * NSH, D:D + 2] = (
            (rmax * (1.0 / 127.0)).astype(np.float16).view(np.int8)
        )
    wall = np.empty((4 * D, D), dtype=np.float16)
    wall[0 * D:1 * D] = np.asarray(Wq).T
    wall[1 * D:2 * D] = np.asarray(Wk).T
    wall[2 * D:3 * D] = np.asarray(Wv).T
    wall[3 * D:4 * D] = np.asarray(Wout).T
    bout_b = (
        np.asarray(bout, dtype=np.float16).reshape(1, 1, D).view(np.int8)
    )
    idxn = np.asarray(idx).astype(np.int32)
    idxt = (idxn + NBC * (idxn // NBC)).astype(np.int16)
    idx_b = idxt.view(np.int8).reshape(NCORES, 64, 2 * D)
    wcomb = np.concatenate(
        [
            wall.view(np.int8).reshape(NCORES, 4 * D // NCORES, 2 * D),
            np.broadcast_to(bout_b, (NCORES, 1, 2 * D)),
            idx_b,
        ],
        axis=1,
    ).reshape(NCORES * (4 * D // NCORES + 1 + 64), 2 * D)
    return {"x_sh": xcomb, "w_sh": wcomb}


def kernel(x, idx, Wq, Wk, Wv, Wout, bout):
    import jax

    sharded, in_names, out_names, sharding = _get_runner()
    devices = _RUNNER_CACHE["devices"]
    # Quantize x to int8 with a per-row scale (exact for Q,K thanks to the
    # per-head LayerNorm's scale invariance; V re-applies the scale on
    # device). Core c owns node range [1024c, 1024c+1024) of both batches.
    # Quantize and upload per-core chunks so the wire streams while the
    # CPU quantizes the next chunk.
    XC = D + 2
    NBC = NSH // B
    xf = np.asarray(x, dtype=np.float32)

    # Small weight|bias|idx tensor first — gets the wire streaming while
    # the CPU quantizes the x chunks below.
    wall = np.empty((4 * D, D), dtype=np.float16)
    wall[0 * D:1 * D] = np.asarray(Wq).T
    wall[1 * D:2 * D] = np.asarray(Wk).T
    wall[2 * D:3 * D] = np.asarray(Wv).T
    wall[3 * D:4 * D] = np.asarray(Wout).T
    bout_b = (
        np.asarray(bout, dtype=np.float16).reshape(1, 1, D).view(np.int8)
    )
    # idx pre-transformed to kv_full batch-0 row ids: n + 1024*(n//1024)
    idxn = np.asarray(idx).astype(np.int32)
    idxt = (idxn + NBC * (idxn // NBC)).astype(np.int16)
    idx_b = idxt.view(np.int8).reshape(NCORES, 64, 2 * D)
    wcomb = np.concatenate(
        [
            wall.view(np.int8).reshape(NCORES, 4 * D // NCORES, 2 * D),
            np.broadcast_to(bout_b, (NCORES, 1, 2 * D)),
            idx_b,
        ],
        axis=1,
    ).reshape(NCORES * (4 * D // NCORES + 1 + 64), 2 * D)
    dw = jax.device_put(wcomb, sharding)

    shards = []
    for c in range(NCORES):
        rows = slice(c * NBC, (c + 1) * NBC)
        xfc = np.concatenate([xf[0, rows], xf[1, rows]], axis=0)
        rmax = np.abs(xfc).max(axis=1, keepdims=True)
        rmax[rmax == 0] = 1.0
        q = np.multiply(xfc, 127.0 / rmax, dtype=np.float32)
        np.rint(q, out=q)
        xc = np.empty((NSH, XC), dtype=np.int8)
        xc[:, 0:D] = q
        xc[:, D:D + 2] = (
            (rmax * (1.0 / 127.0)).astype(np.float16).view(np.int8)
        )
        shards.append(jax.device_put(xc, devices[c]))
    dx = jax.make_array_from_single_device_arrays(
        (NCORES * NSH, XC), sharding, shards
    )

    gin = {"x_sh": dx, "w_sh": dw}
    args = [gin[name] for name in in_names]
    out_arrs = sharded(*args)

    # Fetch output shards in parallel and dequantize each as it lands.
    # Shard c rows: [0:1024) = batch 0 nodes [1024c, 1024c+1024),
    # [1024:2048) = batch 1 same node range.
    from concurrent.futures import ThreadPoolExecutor

    res = np.empty((B, NB, D), dtype=np.float32)

    def fetch_dequant(c_shard):
        c, shard = c_shard
        buf = np.asarray(shard.data)         # (NSH, D+2) int8
        osc = np.ascontiguousarray(buf[:, D:D + 2]).view(np.float16)
        oscf = osc.astype(np.float32)
        rows = slice(c * NBC, (c + 1) * NBC)
        np.multiply(
            buf[0:NBC, 0:D], oscf[0:NBC], dtype=np.float32, out=res[0, rows]
        )
        np.multiply(
            buf[NBC:, 0:D], oscf[NBC:], dtype=np.float32, out=res[1, rows]
        )

    pool = _RUNNER_CACHE.setdefault("pool", ThreadPoolExecutor(NCORES))
    shards_out = sorted(
        out_arrs[0].addressable_shards, key=lambda s: s.index[0].start or 0
    )
    list(pool.map(fetch_dequant, enumerate(shards_out)))
    return res
